# revision 8
# baseline (speedup 1.0000x reference)
"""ANI-style species-routed MLP (MoE routing) on 8 TRN2 NeuronCores.

Strategy:
- Data-parallel over molecules: core c handles molecules [128c, 128(c+1)).
- Host groups each core's 6144 atoms by species (counting sort), pads each
  species bucket to a shared uniform capacity, and ships the aev
  feature-major (transposed, partition-major, fp16) so features sit on SBUF
  partitions and every DMA is 128 long contiguous rows.
- Device computes, per species segment, the dense 4-layer MLP chain with
  fp16 matmuls (f32 PSUM accumulate), CELU via one exact trick:
      celu(x) + 0.1 = min(0.1*exp(10x), 0.1) + relu(x)
  The +0.1 offset is folded into the next layer's bias on the host
  (beta = b - 0.1 * rowsum(W)); the 0.1 output scale of the exp is folded
  into the activation bias (ln(0.1)).
- CELU needs no relu pass: u = max(z + (beta+0.1), min(e', 0.1)) exactly,
  by convexity of exp. Engine split: exp on ACT, min on GPSIMD (SBUF fp16),
  fused bias-add+max on DVE (the only extra PSUM read).
- Final per-molecule reduction on host (cheap), including the per-species
  output bias b4 - 0.1*rowsum(W4).
"""
import os
import sys

sys.path.insert(0, "/opt/trn_rl_repo")

from contextlib import ExitStack

import numpy as np

import concourse.bass as bass
import concourse.mybir as mybir
import concourse.tile as tile
from concourse import bacc
from concourse.bass_utils import run_bass_kernel_spmd

F32 = mybir.dt.float32
F16 = mybir.dt.float16
AF = mybir.ActivationFunctionType
ALU = mybir.AluOpType

B, A, F = 1024, 48, 384
S = 7
NCORES = 8
BM = B // NCORES  # molecules per core
ALPHA = 0.1
LN_ALPHA = float(np.log(ALPHA))

_CACHE = {}
LAST_EXEC_NS = None

# engine for the min(e,0.1) pass: "gpsimd" | "vector"
MIN_ENGINE = "gpsimd"


def _build(cap):
    """SPMD graph: uniform per-species capacity `cap` (atoms, mult of 64)."""
    half = cap // 2
    a_pad = S * cap
    nc = bacc.Bacc()

    xt_d = nc.declare_dram_parameter("xt", [128, S, 3, cap], F16, isOutput=False)
    w1_d = nc.declare_dram_parameter("w1t", [128, S, 3, 256], F16, isOutput=False)
    w2_d = nc.declare_dram_parameter("w2t", [128, S, 2, 192], F16, isOutput=False)
    w3_d = nc.declare_dram_parameter("w3t", [128, S, 2, 160], F16, isOutput=False)
    w4_d = nc.declare_dram_parameter("w4t", [128, S, 2, 1], F16, isOutput=False)
    # biases: [128, species, layer(3), kind(exp/comb), chunk(2)]
    b_d = nc.declare_dram_parameter("biases", [128, S, 3, 2, 2], F32, isOutput=False)
    en_d = nc.declare_dram_parameter("energy", [1, a_pad], F32, isOutput=True)

    l1_m = [(0, 128), (128, 128)]
    l2_m = [(0, 128), (128, 64)]
    l3_m = [(0, 128), (128, 32)]
    l2_k = [(0, 128), (128, 128)]
    l3_k = [(0, 128), (128, 64)]
    l4_k = [(0, 128), (128, 32)]

    with tile.TileContext(nc) as tc, ExitStack() as ctx:
        wpool = ctx.enter_context(tc.tile_pool(name="weights", bufs=1))
        xpool = ctx.enter_context(tc.tile_pool(name="x", bufs=4))
        upool = ctx.enter_context(tc.tile_pool(name="u", bufs=2))
        tpool = ctx.enter_context(tc.tile_pool(name="t", bufs=3))
        zpool = ctx.enter_context(tc.tile_pool(name="z", bufs=3, space="PSUM"))
        z4pool = ctx.enter_context(tc.tile_pool(name="z4", bufs=2, space="PSUM"))
        epool = ctx.enter_context(tc.tile_pool(name="en", bufs=1))

        w1 = wpool.tile([128, S, 3, 256], F16)
        nc.sync.dma_start(w1[:], w1_d.ap())
        w2 = wpool.tile([128, S, 2, 192], F16)
        nc.sync.dma_start(w2[:], w2_d.ap())
        w3 = wpool.tile([128, S, 2, 160], F16)
        nc.sync.dma_start(w3[:], w3_d.ap())
        w4 = wpool.tile([128, S, 2, 1], F16)
        nc.sync.dma_start(w4[:], w4_d.ap())
        bb = wpool.tile([128, S, 3, 2, 2], F32)
        nc.sync.dma_start(bb[:], b_d.ap())

        en_sb = epool.tile([1, a_pad], F32)

        chunk_idx = 0

        def celu(z, u_out, s, l, m):
            nonlocal chunk_idx
            p = z.shape[0]
            n = z.shape[-1]
            bx = bb[:p, s, l - 1, 0, m : m + 1]
            bc = bb[:p, s, l - 1, 1, m : m + 1]
            e = tpool.tile([128, cap], F16, tag="e")
            nc.scalar.activation(e[:p, :n], z[:], AF.Exp, bias=bx, scale=10.0)
            mt = tpool.tile([128, cap], F16, tag="mt")
            eng = nc.gpsimd if MIN_ENGINE == "gpsimd" else nc.vector
            eng.tensor_scalar(mt[:p, :n], e[:p, :n], ALPHA, None, op0=ALU.min)
            chunk_idx += 1
            nc.vector.scalar_tensor_tensor(
                u_out[:], z[:], bc, mt[:p, :n], op0=ALU.add, op1=ALU.max
            )

        for s in range(S):
            x = xpool.tile([128, 3, cap], F16, tag="x")
            nc.sync.dma_start(x[:], xt_d.ap()[:, s])
            u1 = upool.tile([128, 2, cap], F16, tag="u1")
            for mi, (mo, mw) in enumerate(l1_m):
                z = zpool.tile([128, cap], F32, tag="z")
                for h in range(2):
                    hs = slice(h * half, (h + 1) * half)
                    for k in range(3):
                        nc.tensor.matmul(
                            z[:mw, hs],
                            w1[:, s, k, mo : mo + mw],
                            x[:, k, hs],
                            start=(k == 0),
                            stop=(k == 2),
                        )
                celu(z[:mw, :], u1[:mw, mi, :], s, 1, mi)
            u2 = upool.tile([128, 2, cap], F16, tag="u2")
            for mi, (mo, mw) in enumerate(l2_m):
                z = zpool.tile([128, cap], F32, tag="z")
                for h in range(2):
                    hs = slice(h * half, (h + 1) * half)
                    for ki, (ko, kw) in enumerate(l2_k):
                        nc.tensor.matmul(
                            z[:mw, hs],
                            w2[:kw, s, ki, mo : mo + mw],
                            u1[:kw, ki, hs],
                            start=(ki == 0),
                            stop=(ki == 1),
                        )
                celu(z[:mw, :], u2[:mw, mi, :], s, 2, mi)
            u3 = upool.tile([128, 2, cap], F16, tag="u3")
            for mi, (mo, mw) in enumerate(l3_m):
                z = zpool.tile([128, cap], F32, tag="z")
                for h in range(2):
                    hs = slice(h * half, (h + 1) * half)
                    for ki, (ko, kw) in enumerate(l3_k):
                        nc.tensor.matmul(
                            z[:mw, hs],
                            w3[:kw, s, ki, mo : mo + mw],
                            u2[:kw, ki, hs],
                            start=(ki == 0),
                            stop=(ki == 1),
                        )
                celu(z[:mw, :], u3[:mw, mi, :], s, 3, mi)
            for h in range(2):
                hs = slice(h * half, (h + 1) * half)
                row = 2 * s + h
                z4 = z4pool.tile([1, half], F32, tag="z4")
                for ki, (ko, kw) in enumerate(l4_k):
                    nc.tensor.matmul(
                        z4[:],
                        w4[:kw, s, ki, 0:1],
                        u3[:kw, ki, hs],
                        start=(ki == 0),
                        stop=(ki == 1),
                    )
                oo = row * half
                if row % 2 == 0:
                    nc.vector.tensor_copy(en_sb[0:1, oo : oo + half], z4[:])
                else:
                    nc.scalar.activation(en_sb[0:1, oo : oo + half], z4[:], AF.Copy)

        nc.sync.dma_start(en_d.ap(), en_sb[:])

    nc.compile()
    return nc


def _to_pmajor(wt, k_pad):
    """[S, M, K] weights -> [128, S, k_pad//128, M] fp16 partition-major."""
    s, m, k = wt.shape
    arr = np.zeros((s, m, k_pad), np.float32)
    arr[:, :, :k] = wt
    out = arr.transpose(2, 0, 1).reshape(k_pad // 128, 128, s, m).transpose(1, 2, 0, 3)
    return np.ascontiguousarray(out, dtype=np.float16)


def _prep_weights(W1, b1, W2, b2, W3, b3, W4, b4):
    beta1 = b1
    beta2 = b2 - ALPHA * W2.sum(axis=2)
    beta3 = b3 - ALPHA * W3.sum(axis=2)
    ec = (b4[:, 0] - ALPHA * W4[:, 0, :].sum(axis=1)).astype(np.float32)

    biases = np.zeros((S, 3, 2, 2, 128), np.float32)
    for li, beta in enumerate((beta1, beta2, beta3)):
        m = beta.shape[1]
        bx = np.zeros((S, 256), np.float32)
        br = np.zeros((S, 256), np.float32)
        bx[:, :m] = 10.0 * beta + LN_ALPHA
        br[:, :m] = beta + ALPHA
        biases[:, li, 0] = bx.reshape(S, 2, 128)
        biases[:, li, 1] = br.reshape(S, 2, 128)
    biases_p = np.ascontiguousarray(biases.transpose(4, 0, 1, 2, 3))

    return dict(
        w1t=_to_pmajor(W1, 384),
        w2t=_to_pmajor(W2, 256),
        w3t=_to_pmajor(W3, 256),
        w4t=_to_pmajor(W4, 256),
        biases=biases_p,
    ), ec


def kernel(species, aev, W1, b1, W2, b2, W3, b3, W4, b4):
    global LAST_EXEC_NS
    species = np.asarray(species)
    aev = np.asarray(aev, dtype=np.float32)
    args = [np.asarray(x, dtype=np.float32)
            for x in (W1, b1, W2, b2, W3, b3, W4, b4)]
    wp, ec = _prep_weights(*args)

    # --- host routing: per-core counting sort by species ---
    sp_c = species.reshape(NCORES, BM * A)
    counts = np.stack([np.bincount(sp_c[c], minlength=S) for c in range(NCORES)])
    cap = int(((counts.max() + 63) // 64) * 64)
    cap = max(cap, 128)

    if cap not in _CACHE:
        _CACHE[cap] = _build(cap)
    nc = _CACHE[cap]

    aev_c = aev.reshape(NCORES, BM * A, F)
    in_maps = []
    perms = []
    for c in range(NCORES):
        perm = np.argsort(sp_c[c], kind="stable")
        perms.append(perm)
        xt = np.zeros((128, S, 3, cap), np.float16)
        pos = 0
        for s in range(S):
            n = counts[c, s]
            blk = aev_c[c][perm[pos : pos + n]].T.astype(np.float16)  # [384, n]
            xt[:, s, :, :n] = blk.reshape(3, 128, n).transpose(1, 0, 2)
            pos += n
        in_maps.append({"xt": xt, **wp})

    trace = bool(os.environ.get("KERNEL_TRACE"))
    res = run_bass_kernel_spmd(nc, in_maps, list(range(NCORES)), trace=trace)
    LAST_EXEC_NS = res.exec_time_ns

    # --- host reduction: scatter atom energies back to molecules ---
    out = np.zeros((NCORES, BM), np.float64)
    for c in range(NCORES):
        en = np.asarray(res.results[c]["energy"][0], np.float64)
        atom_e = np.empty(BM * A, np.float64)
        pos = 0
        for s in range(S):
            n = counts[c, s]
            atom_e[perms[c][pos : pos + n]] = en[s * cap : s * cap + n]
            pos += n
        out[c] = atom_e.reshape(BM, A).sum(axis=1)
        out[c] += np.asarray(ec, np.float64)[sp_c[c]].reshape(BM, A).sum(axis=1)
    return out.reshape(B).astype(np.float32)


# revision 15
# speedup vs baseline: 5.0077x; 5.0077x over previous
"""ANI-style species-routed MLP (MoE routing) on 8 TRN2 NeuronCores.

Strategy:
- Data-parallel over molecules: core c handles molecules [128c, 128(c+1)).
- Host groups each core's 6144 atoms by species (counting sort), pads each
  species bucket to a shared uniform capacity, and ships the aev
  feature-major (transposed, partition-major, fp16) so features sit on SBUF
  partitions and every DMA is 128 long contiguous rows.
- Device computes, per species segment, the dense 4-layer MLP chain with
  fp16 matmuls (f32 PSUM accumulate), CELU via one exact trick:
      celu(x) + 0.1 = min(0.1*exp(10x), 0.1) + relu(x)
  The +0.1 offset is folded into the next layer's bias on the host
  (beta = b - 0.1 * rowsum(W)); the 0.1 output scale of the exp is folded
  into the activation bias (ln(0.1)).
- CELU needs no relu pass: u = max(z + (beta+0.1), min(e', 0.1)) exactly,
  by convexity of exp. Engine split: exp on ACT, min on GPSIMD (SBUF fp16),
  fused bias-add+max on DVE (the only extra PSUM read).
- Final per-molecule reduction on host (cheap), including the per-species
  output bias b4 - 0.1*rowsum(W4).
"""
import os
import sys

sys.path.insert(0, "/opt/trn_rl_repo")

from contextlib import ExitStack

import numpy as np

import concourse.bass as bass
import concourse.mybir as mybir
import concourse.tile as tile
from concourse import bacc
from concourse.bass_utils import run_bass_kernel_spmd

F32 = mybir.dt.float32
F16 = mybir.dt.float16
AF = mybir.ActivationFunctionType
ALU = mybir.AluOpType

B, A, F = 1024, 48, 384
S = 7
NCORES = 8
BM = B // NCORES  # molecules per core
ALPHA = 0.1
LN_ALPHA = float(np.log(ALPHA))

_CACHE = {}
LAST_EXEC_NS = None

# celu chunks with idx % MOD == PHASE use the relu-form (ACT-heavy):
#   u = stt(e, 0.1, r) = min(e,0.1)+relu(z+beta)   [ACT: exp+relu, DVE: stt]
# others use the max-form (DVE-heavy):
#   u = max(z+(beta+0.1), min(e,0.1))              [ACT: exp, DVE: min+combine]
RELU_FORM_SET = frozenset({5, 6, 7})  # of idx % 8


def _build(cap):
    """SPMD graph: uniform per-species capacity `cap` (atoms, mult of 512 so
    every matmul output tile is PSUM-bank aligned)."""
    assert cap % 512 == 0
    half = 512
    n_half = cap // 512
    a_pad = S * cap
    nc = bacc.Bacc()

    xt_d = nc.declare_dram_parameter("xt", [128, S, 3, cap], F16, isOutput=False)
    w1_d = nc.declare_dram_parameter("w1t", [128, S, 3, 256], F16, isOutput=False)
    w2_d = nc.declare_dram_parameter("w2t", [128, S, 2, 192], F16, isOutput=False)
    w3_d = nc.declare_dram_parameter("w3t", [128, S, 2, 160], F16, isOutput=False)
    w4_d = nc.declare_dram_parameter("w4t", [128, S, 2, 1], F16, isOutput=False)
    # biases: [128, species, layer(3), kind(exp/comb/relu), chunk(2)]
    b_d = nc.declare_dram_parameter("biases", [128, S, 3, 3, 2], F32, isOutput=False)
    en_d = nc.declare_dram_parameter("energy", [1, a_pad], F32, isOutput=True)

    l1_m = [(0, 128), (128, 128)]
    l2_m = [(0, 128), (128, 64)]
    l3_m = [(0, 128), (128, 32)]
    l2_k = [(0, 128), (128, 128)]
    l3_k = [(0, 128), (128, 64)]
    l4_k = [(0, 128), (128, 32)]

    with tile.TileContext(nc) as tc, ExitStack() as ctx:
        wpool = ctx.enter_context(tc.tile_pool(name="weights", bufs=1))
        xpool = ctx.enter_context(tc.tile_pool(name="x", bufs=4))
        upool = ctx.enter_context(tc.tile_pool(name="u", bufs=3))
        tpool = ctx.enter_context(tc.tile_pool(name="t", bufs=4))
        zpool = ctx.enter_context(tc.tile_pool(name="z", bufs=3, space="PSUM"))
        z4pool = ctx.enter_context(tc.tile_pool(name="z4", bufs=2, space="PSUM"))
        epool = ctx.enter_context(tc.tile_pool(name="en", bufs=1))

        w1 = wpool.tile([128, S, 3, 256], F16)
        nc.sync.dma_start(w1[:], w1_d.ap())
        w2 = wpool.tile([128, S, 2, 192], F16)
        nc.sync.dma_start(w2[:], w2_d.ap())
        w3 = wpool.tile([128, S, 2, 160], F16)
        nc.sync.dma_start(w3[:], w3_d.ap())
        w4 = wpool.tile([128, S, 2, 1], F16)
        nc.sync.dma_start(w4[:], w4_d.ap())
        bb = wpool.tile([128, S, 3, 3, 2], F32)
        nc.sync.dma_start(bb[:], b_d.ap())

        en_sb = epool.tile([1, a_pad], F32)

        chunk_idx = 0

        def celu(z, u_out, s, l, m):
            nonlocal chunk_idx
            p = z.shape[0]
            n = z.shape[-1]
            bx = bb[:p, s, l - 1, 0, m : m + 1]
            bc = bb[:p, s, l - 1, 1, m : m + 1]
            br = bb[:p, s, l - 1, 2, m : m + 1]
            e = tpool.tile([128, cap], F16, tag="e")
            nc.scalar.activation(e[:p, :n], z[:], AF.Exp, bias=bx, scale=10.0)
            if chunk_idx % 8 in RELU_FORM_SET:
                r = tpool.tile([128, cap], F16, tag="r")
                nc.scalar.activation(r[:p, :n], z[:], AF.Relu, bias=br, scale=1.0)
                nc.vector.scalar_tensor_tensor(
                    u_out[:], e[:p, :n], ALPHA, r[:p, :n],
                    op0=ALU.min, op1=ALU.add,
                )
            else:
                mt = tpool.tile([128, cap], F16, tag="mt")
                nc.vector.tensor_scalar(
                    mt[:p, :n], e[:p, :n], ALPHA, None, op0=ALU.min
                )
                nc.vector.scalar_tensor_tensor(
                    u_out[:], z[:], bc, mt[:p, :n], op0=ALU.add, op1=ALU.max
                )
            chunk_idx += 1

        for s in range(S):
            x = xpool.tile([128, 3, cap], F16, tag="x")
            nc.sync.dma_start(x[:], xt_d.ap()[:, s])
            u1 = upool.tile([128, 2, cap], F16, tag="u1")
            for mi, (mo, mw) in enumerate(l1_m):
                z = zpool.tile([128, cap], F32, tag="z")
                for h in range(n_half):
                    hs = slice(h * half, (h + 1) * half)
                    for k in range(3):
                        nc.tensor.matmul(
                            z[:mw, hs],
                            w1[:, s, k, mo : mo + mw],
                            x[:, k, hs],
                            start=(k == 0),
                            stop=(k == 2),
                        )
                celu(z[:mw, :], u1[:mw, mi, :], s, 1, mi)
            u2 = upool.tile([128, 2, cap], F16, tag="u2")
            for mi, (mo, mw) in enumerate(l2_m):
                z = zpool.tile([128, cap], F32, tag="z")
                for h in range(n_half):
                    hs = slice(h * half, (h + 1) * half)
                    for ki, (ko, kw) in enumerate(l2_k):
                        nc.tensor.matmul(
                            z[:mw, hs],
                            w2[:kw, s, ki, mo : mo + mw],
                            u1[:kw, ki, hs],
                            start=(ki == 0),
                            stop=(ki == 1),
                        )
                celu(z[:mw, :], u2[:mw, mi, :], s, 2, mi)
            u3 = upool.tile([128, 2, cap], F16, tag="u3")
            for mi, (mo, mw) in enumerate(l3_m):
                z = zpool.tile([128, cap], F32, tag="z")
                for h in range(n_half):
                    hs = slice(h * half, (h + 1) * half)
                    for ki, (ko, kw) in enumerate(l3_k):
                        nc.tensor.matmul(
                            z[:mw, hs],
                            w3[:kw, s, ki, mo : mo + mw],
                            u2[:kw, ki, hs],
                            start=(ki == 0),
                            stop=(ki == 1),
                        )
                celu(z[:mw, :], u3[:mw, mi, :], s, 3, mi)
            for h in range(n_half):
                hs = slice(h * half, (h + 1) * half)
                row = n_half * s + h
                z4 = z4pool.tile([1, half], F32, tag="z4")
                for ki, (ko, kw) in enumerate(l4_k):
                    nc.tensor.matmul(
                        z4[:],
                        w4[:kw, s, ki, 0:1],
                        u3[:kw, ki, hs],
                        start=(ki == 0),
                        stop=(ki == 1),
                    )
                oo = row * half
                nc.vector.tensor_copy(en_sb[0:1, oo : oo + half], z4[:])

        nc.sync.dma_start(en_d.ap(), en_sb[:])

    nc.compile()
    return nc


def _to_pmajor(wt, k_pad):
    """[S, M, K] weights -> [128, S, k_pad//128, M] fp16 partition-major."""
    s, m, k = wt.shape
    arr = np.zeros((s, m, k_pad), np.float32)
    arr[:, :, :k] = wt
    out = arr.transpose(2, 0, 1).reshape(k_pad // 128, 128, s, m).transpose(1, 2, 0, 3)
    return np.ascontiguousarray(out, dtype=np.float16)


def _prep_weights(W1, b1, W2, b2, W3, b3, W4, b4):
    beta1 = b1
    beta2 = b2 - ALPHA * W2.sum(axis=2)
    beta3 = b3 - ALPHA * W3.sum(axis=2)
    ec = (b4[:, 0] - ALPHA * W4[:, 0, :].sum(axis=1)).astype(np.float32)

    biases = np.zeros((S, 3, 3, 2, 128), np.float32)
    for li, beta in enumerate((beta1, beta2, beta3)):
        m = beta.shape[1]
        bx = np.zeros((S, 256), np.float32)
        bc = np.zeros((S, 256), np.float32)
        br = np.zeros((S, 256), np.float32)
        bx[:, :m] = 10.0 * beta + LN_ALPHA
        bc[:, :m] = beta + ALPHA
        br[:, :m] = beta
        biases[:, li, 0] = bx.reshape(S, 2, 128)
        biases[:, li, 1] = bc.reshape(S, 2, 128)
        biases[:, li, 2] = br.reshape(S, 2, 128)
    biases_p = np.ascontiguousarray(biases.transpose(4, 0, 1, 2, 3))

    return dict(
        w1t=_to_pmajor(W1, 384),
        w2t=_to_pmajor(W2, 256),
        w3t=_to_pmajor(W3, 256),
        w4t=_to_pmajor(W4, 256),
        biases=biases_p,
    ), ec


def kernel(species, aev, W1, b1, W2, b2, W3, b3, W4, b4):
    global LAST_EXEC_NS
    species = np.asarray(species)
    aev = np.asarray(aev, dtype=np.float32)
    args = [np.asarray(x, dtype=np.float32)
            for x in (W1, b1, W2, b2, W3, b3, W4, b4)]
    wp, ec = _prep_weights(*args)

    # --- host routing: per-core counting sort by species ---
    sp_c = species.reshape(NCORES, BM * A)
    counts = np.stack([np.bincount(sp_c[c], minlength=S) for c in range(NCORES)])
    cap = int(((counts.max() + 511) // 512) * 512)

    if cap not in _CACHE:
        _CACHE[cap] = _build(cap)
    nc = _CACHE[cap]

    aev_c = aev.reshape(NCORES, BM * A, F)
    in_maps = []
    perms = []
    for c in range(NCORES):
        perm = np.argsort(sp_c[c], kind="stable")
        perms.append(perm)
        xt = np.zeros((128, S, 3, cap), np.float16)
        pos = 0
        for s in range(S):
            n = counts[c, s]
            blk = aev_c[c][perm[pos : pos + n]].T.astype(np.float16)  # [384, n]
            xt[:, s, :, :n] = blk.reshape(3, 128, n).transpose(1, 0, 2)
            pos += n
        in_maps.append({"xt": xt, **wp})

    trace = bool(os.environ.get("KERNEL_TRACE"))
    res = run_bass_kernel_spmd(nc, in_maps, list(range(NCORES)), trace=trace)
    LAST_EXEC_NS = res.exec_time_ns

    # --- host reduction: scatter atom energies back to molecules ---
    out = np.zeros((NCORES, BM), np.float64)
    for c in range(NCORES):
        en = np.asarray(res.results[c]["energy"][0], np.float64)
        atom_e = np.empty(BM * A, np.float64)
        pos = 0
        for s in range(S):
            n = counts[c, s]
            atom_e[perms[c][pos : pos + n]] = en[s * cap : s * cap + n]
            pos += n
        out[c] = atom_e.reshape(BM, A).sum(axis=1)
        out[c] += np.asarray(ec, np.float64)[sp_c[c]].reshape(BM, A).sum(axis=1)
    return out.reshape(B).astype(np.float32)
